# revision 51
# baseline (speedup 1.0000x reference)
"""GAT message-passing kernel for Trainium2 (8 NeuronCores, data-parallel over batch).

Math (per batch element b, derived from the reference nn.Module):
    x      = nodes.reshape(N, D)
    self_e = mlp2(x, self_*)                 # [N, H]
    nb_e   = mlp2(x, nb_*)                   # [N, H]
    U      = self_e @ comb_w1[:H]            # [N, H]  (i side)
    V      = nb_e @ comb_w1[H:] + comb_b1    # [N, H]  (j side)
    scores(i,j) = leaky(U_i + V_j) @ w2 + b2
                = 0.8*relu(U_i+V_j)@w2 + 0.2*(sU_i + sV_j) + const_i
    Softmax over j drops per-i constants; exp factorizes as
      E^T[j,i] = edges[j,i]*(j!=i)*exp(0.2 sV_j) * exp(0.8 relu(U_i+V_j)@w2)
    denom[i] = sum_j E^T[j,i]; gate = denom > eps
    out[i]   = gate * ((gate/denom) * (E^T)^T-free agg + self_e)
    (|scores| < ~3, so exp needs no max-subtraction.)

Device mapping (one core per batch element). Differences vs the 76us baseline:
  - j-major pairwise layout: build tiles are [(j-parity g, h) partitions, i free]
    so scores land in PSUM already transposed to the [j, i] layout the denom /
    aggregation matmuls need -> zero PE transposes of the score tiles.
  - Mixed-precision score reduction: per 32-slot column group, the first B2G
    slots are bf16 tiles built by DVE (4x mode, ~239ns) and consumed by normal
    bf16 matmuls (~213ns); the rest are fp8e4m3 tiles built mostly by ACT/Pool
    (dtype-indifferent engines) and consumed two-at-a-time by DoubleRow
    matmuls (~107ns per pair -> 4x PE throughput).
  - Preamble MLP matmuls use float32r moving operands (1 cycle/row at free>=512
    vs 4 for fp32, exact fp32 numerics in CoreSim).
  - denom is folded into the aggregation matmul by appending a ones column to
    nb_e (rhs [nbe | 1], free=65): denom lands per-i-partition in PSUM, so no
    row->column transpose is needed.
  - self_e / nb_e row-major tiles come from PE transposes of eT_s / eT_n
    (f32r, 1.5 cyc/row) instead of extra matmuls.
"""

import os
import sys

sys.path.insert(0, "/opt/trn_rl_repo")

import numpy as np
import ml_dtypes

import concourse.bass as bass
import concourse.bacc as bacc
import concourse.tile as tile
from concourse import mybir, bass2jax
from concourse.bass_utils import run_bass_kernel_spmd

B, N, H, D = 8, 512, 64, 128
NCORES = 8
NT = N // 128          # 4 j/i tiles of 128
F32 = mybir.dt.float32
F32R = mybir.dt.float32r
BF16 = mybir.dt.bfloat16
FP8 = mybir.dt.float8e4
DR = mybir.MatmulPerfMode.DoubleRow

# bf16 slots per 32-slot column group (rest are fp8 DoubleRow pairs).
B2G = int(os.environ.get("GAT_B2G", "16"))
assert B2G % 2 == 0 and 0 <= B2G <= 32
NDR = (32 - B2G) // 2   # DoubleRow pairs per group

# Engine schedule for fp8 build halves: 'a' = ScalarE, 'p' = gpsimd, 'v' = DVE.
def _default_fp8_pattern():
    pat = []
    for q in range(16):
        if q == 0:
            # first pair of each j-tile on DVE+Pool: ACT's queue is the
            # late one when the pipeline (re)fills
            pat += ["v", "p"]
        elif q % 8 == 7:
            pat += ["p", "v"]
        elif q % 4 == 3 or q == 8:
            pat += ["p", "p"]
        else:
            pat += ["a", "p"]
    return "".join(pat)

FP8_PATTERN = os.environ.get("GAT_FP8_PATTERN", _default_fp8_pattern())

_CACHE = {}


def _build_module():
    nc = bacc.Bacc("TRN2", target_bir_lowering=False, debug=False, num_devices=NCORES)

    # ---- per-core data ----
    nodes = nc.dram_tensor("nodes", [N, D], F32R, kind="ExternalInput")
    edges = nc.dram_tensor("edges", [N, N], mybir.dt.uint8, kind="ExternalInput")
    # ---- parameters / host-prepared constants (same on all cores) ----
    # wpack1 = w1_self | w1_nb | id128  (f32r, 128 partitions)
    wpack1 = nc.dram_tensor("wpack1", [128, 256], F32R, kind="ExternalInput")
    # wpack2 = w2_self | w2_nb | [w1_cs|w1_cs] | w1_cn | wcv  (f32r, 64 parts)
    wpack2 = nc.dram_tensor("wpack2", [H, 321], F32R, kind="ExternalInput")
    bvec = nc.dram_tensor("bvec", [128, 7], F32, kind="ExternalInput")
    w2bdpack = nc.dram_tensor("w2bdpack", [128, 2], BF16, kind="ExternalInput")
    f8lhs = nc.dram_tensor("f8lhs", [128, max((64 - 2 * B2G) // 2, 1), 2, H], FP8,
                           kind="ExternalInput")
    out = nc.dram_tensor("out", [N, H], F32, kind="ExternalOutput")

    with tile.TileContext(nc) as tc:
        _emit(nc, tc, locals())
    nc.compile()
    return nc


def _emit(nc, tc, t):
    AF = mybir.ActivationFunctionType
    OP = mybir.AluOpType

    with (
        tc.tile_pool(name="persist", bufs=1) as P,
        tc.tile_pool(name="ework", bufs=2) as EW,
        tc.tile_pool(name="relu", bufs=34) as RL,
        tc.tile_pool(name="relup", bufs=18) as RP,
        tc.tile_pool(name="xexp", bufs=3) as XE,
        tc.tile_pool(name="small", bufs=6) as SM,
        tc.tile_pool(name="psumR", bufs=2, space="PSUM") as PR,
        tc.tile_pool(name="psumT", bufs=3, space="PSUM") as PT,
        tc.tile_pool(name="psumM", bufs=2, space="PSUM") as PM,
        tc.tile_pool(name="psumA", bufs=1, space="PSUM") as PA,
    ):
        ts = bass.ts

        # ---------- input DMAs, spread across engine queues ----------
        # x^T source: xall[p, 128*it + d] = nodes[128*it + p, d]
        xall = P.tile([128, N], F32R, tag="xall")
        nsrc = t["nodes"].ap().rearrange("(t p) d -> p t d", t=NT)
        xdst = xall[:].rearrange("p (t d) -> p t d", t=NT)
        nc.sync.dma_start(out=xdst[:, 0:2], in_=nsrc[:, 0:2])
        nc.sync.dma_start(out=xdst[:, 2:4], in_=nsrc[:, 2:4])
        wp1 = P.tile([128, 256], F32R, tag="wp1")
        nc.scalar.dma_start(out=wp1[:, 128:256], in_=t["wpack1"].ap()[:, 128:256])
        nc.scalar.dma_start(out=wp1[:, 0:128], in_=t["wpack1"].ap()[:, 0:128])
        w1s, w1n, idf = wp1[:, 0:H], wp1[:, H:2 * H], wp1[:, 128:256]
        idf1 = P.tile([1, 1], F32, tag="idf1")
        nc.gpsimd.memset(idf1[:], 1.0)
        # wpack2 = [wU|wU] | wV | wsv | w2s | w2n  (f32r, 64 partitions)
        wp2 = P.tile([H, 321], F32R, tag="wp2")
        nc.sync.dma_start(out=wp2[:], in_=t["wpack2"].ap())
        wUd, wV = wp2[:, 0:128], wp2[:, 128:192]
        wsv, w2s, w2n = wp2[:, 192:193], wp2[:, 193:257], wp2[:, 257:321]
        bvec = P.tile([128, 7], F32, tag="bvec")
        nc.gpsimd.dma_start(out=bvec[:], in_=t["bvec"].ap())
        b1s, b1n = bvec[:H, 0:1], bvec[:H, 1:2]
        b2sc, b2nc = bvec[:H, 2:3], bvec[:H, 3:4]
        bU2, bV2 = bvec[:, 4:5], bvec[:, 5:6]
        svbias = bvec[0:1, 6:7]
        w2bd_all = P.tile([128, 128], BF16, tag="w2bd_all")
        nc.gpsimd.memset(w2bd_all[:], 0.0)
        nc.gpsimd.dma_start(out=w2bd_all[:, 62:64], in_=t["w2bdpack"].ap())
        w2bd_sb = [w2bd_all[:, 62 - 2 * s:126 - 2 * s] for s in range(32)]
        f8w = P.tile([128, max((64 - 2 * B2G) // 2, 1), 2, H], FP8, tag="f8w")
        nc.gpsimd.dma_start(out=f8w[:], in_=t["f8lhs"].ap())
        esbs = []
        for jt in range(NT):
            esb = P.tile([128, N], mybir.dt.uint8, tag=f"esb{jt}", name=f"esb{jt}")
            nc.sync.dma_start(out=esb[:], in_=t["edges"].ap()[ts(jt, 128), :])
            esbs.append(esb)

        # ---------- PE warmup: ramp the pstate clock while DMAs land ----------
        NWARM = int(os.environ.get("GAT_NWARM", "0"))
        if NWARM:
            wrm = EW.tile([128, N], BF16, tag="wrm", name="wrm")
            nc.gpsimd.memset(wrm[:], 0.0)
            pw = PR.tile([128, N], F32, tag="psumR", name="pw")
            for i in range(NWARM):
                nc.tensor.matmul(pw[:64, :], wrm[:, 0:64], wrm[:],
                                 start=(i == 0), stop=(i == NWARM - 1))

        # ---------- pipelined preamble: 2 column-halves flow through
        # xpose -> w1 -> leaky -> U/V independently ----------
        xT = P.tile([D, N], F32R, tag="xT")
        pm_s1 = PM.tile([128, N], F32, tag="mm", name="pm_s1")
        pm_n1 = PM.tile([128, N], F32, tag="mm", name="pm_n1")
        h1T_s = P.tile([H, N], F32R, tag="h1T_s")
        h1T_n = P.tile([H, N], F32R, tag="h1T_n")
        pm_u = PM.tile([128, N], F32, tag="mm", name="pm_u")
        pm_v = PM.tile([128, N], F32, tag="mm", name="pm_v")
        Urep = P.tile([128, N], BF16, tag="Urep")
        V2 = P.tile([128, N // 2], F32, tag="V2")
        vsplit = pm_v[:H, :].rearrange("p (j g) -> p j g", g=2)
        for ch in range(2):
            cs = ts(ch, 256)
            for it in (2 * ch, 2 * ch + 1):
                px = PT.tile([128, 128], F32R, tag="pt", name="px",
                             padded_shape=[128, 128])
                nc.tensor.transpose(px[:], xall[:, ts(it, 128)], idf)
                if it % 2 == 0:
                    nc.vector.tensor_copy(out=xT[:, ts(it, 128)], in_=px[:])
                else:
                    nc.scalar.activation(out=xT[:, ts(it, 128)], in_=px[:],
                                         func=AF.Identity, scale=1.0)
            nc.tensor.matmul(pm_s1[:H, cs], w1s, xT[:, cs], start=True,
                             stop=True, skip_group_check=True)
            nc.tensor.matmul(pm_n1[:H, cs], w1n, xT[:, cs], start=True,
                             stop=True, skip_group_check=True)
            # leaky(z) = max(0.2 z, z); z = w1 @ x + b1
            z_s = EW.tile([H, 256], F32R, tag="z_s", name="z_s")
            nc.scalar.activation(out=z_s[:], in_=pm_s1[:H, cs],
                                 func=AF.Identity, bias=b1s, scale=1.0)
            nc.vector.scalar_tensor_tensor(out=h1T_s[:, cs], in0=z_s[:],
                                           scalar=0.2, in1=z_s[:],
                                           op0=OP.mult, op1=OP.max)
            z_n = EW.tile([H, 256], F32R, tag="z_n", name="z_n")
            nc.scalar.activation(out=z_n[:], in_=pm_n1[:H, cs],
                                 func=AF.Identity, bias=b1n, scale=1.0)
            nc.vector.scalar_tensor_tensor(out=h1T_n[:, cs], in0=z_n[:],
                                           scalar=0.2, in1=z_n[:],
                                           op0=OP.mult, op1=OP.max)
            # U duplicated to both partition halves ([wU|wU]); V in j-pair form
            nc.tensor.matmul(pm_u[:, cs], wUd, h1T_s[:, cs], start=True,
                             stop=True, skip_group_check=True)
            nc.tensor.matmul(pm_v[:H, cs], wV, h1T_n[:, cs], start=True,
                             stop=True, skip_group_check=True)
            nc.scalar.activation(out=Urep[:, cs], in_=pm_u[:, cs],
                                 func=AF.Identity, bias=bU2, scale=1.0)
            vcs = slice(128 * ch, 128 * ch + 128)
            nc.scalar.activation(out=V2[:H, vcs], in_=vsplit[:, vcs, 0],
                                 func=AF.Identity, bias=bV2[:H], scale=1.0)
            nc.vector.tensor_scalar_add(out=V2[H:, vcs],
                                        in0=vsplit[:, vcs, 1],
                                        scalar1=bV2[:H])

        esv = P.tile([128, NT], F32, tag="esv")

        def emit_sv():
            # exp(0.2 sV_j) -> [128, NT] per-partition scalars; emitted after
            # jt0's builds so it doesn't clog ACT's head queue
            pm_sv = PM.tile([128, N], F32, tag="mm", name="pm_sv")
            nc.tensor.matmul(pm_sv[:1, :], wsv, h1T_n[:], start=True, stop=True)
            sv_row = SM.tile([1, N], F32, tag="sv_row")
            nc.scalar.activation(out=sv_row[:], in_=pm_sv[:1, :], func=AF.Exp,
                                 bias=svbias, scale=0.2)
            pesv = PT.tile([128, 128], F32, tag="pt", name="pesv",
                           padded_shape=[128, 128])
            for tq in range(NT):
                nc.tensor.transpose(pesv[:, tq:tq + 1], sv_row[:, ts(tq, 128)],
                                    idf1[:])
            nc.vector.tensor_copy(out=esv[:], in_=pesv[:, 0:NT])

        # ---------- deferred: self_e / nb_e row tiles (emitted pre-jt3) ----------
        selfe, nbe65 = [], []

        def emit_enb():
            pm_es = PM.tile([128, N], F32, tag="mm", name="pm_es")
            nc.tensor.matmul(pm_es[:H, :], w2s, h1T_s[:], start=True, stop=True)
            eT_s = P.tile([H, N], F32R, tag="eT_s")
            nc.vector.tensor_scalar_add(out=eT_s[:], in0=pm_es[:H, :],
                                        scalar1=b2sc)
            pm_en = PM.tile([128, N], F32, tag="mm", name="pm_en")
            nc.tensor.matmul(pm_en[:H, :], w2n, h1T_n[:], start=True, stop=True)
            eT_n = P.tile([H, N], F32R, tag="eT_n")
            nc.scalar.activation(out=eT_n[:], in_=pm_en[:H, :], func=AF.Identity,
                                 bias=b2nc, scale=1.0)
            for it in range(NT):
                pse = PT.tile([128, 128], F32R, tag="pt", name="pse",
                              padded_shape=[128, 128])
                nc.tensor.transpose(pse[:, 0:H], eT_s[:, ts(it, 128)],
                                    idf[0:H, 0:H])
                se = P.tile([128, H], F32, tag=f"selfe{it}", name=f"selfe{it}")
                nc.vector.tensor_copy(out=se[:], in_=pse[:, 0:H])
                selfe.append(se)
            for jt in range(NT):
                pne = PT.tile([128, 128], F32R, tag="pt", name="pne",
                              padded_shape=[128, 128])
                nc.tensor.transpose(pne[:, 0:H], eT_n[:, ts(jt, 128)],
                                    idf[0:H, 0:H])
                ne = P.tile([128, H + 8], BF16, tag=f"nbe{jt}", name=f"nbe{jt}")
                nc.vector.tensor_copy(out=ne[:, 0:H], in_=pne[:, 0:H])
                nc.gpsimd.memset(ne[:, H:H + 1], 1.0)
                nbe65.append(ne)

        # ---------- main pass over j-tiles ----------
        pat = FP8_PATTERN
        _pi = [0]
        pa_all = PA.tile([128, NT, H + 1], F32, tag="pa_all")
        et3 = P.tile([128, N], BF16, tag="et3", name="et3")
        ets = []
        for jt in range(NT):
            if jt == NT - 1:
                emit_enb()
            ps = PR.tile([128, N], F32, tag="psumR", name="ps")

            # group c=0 holds every fp8 slot (DoubleRow requires PSUM
            # column-position 0); group c=1 is all-bf16. DR matmuls are
            # interleaved into c1's DVE-paced bf16 stream to keep PE full.
            NDR2 = (64 - 2 * B2G) // 2      # DR pairs, all in c0
            C0BF = 2 * B2G - 32             # leftover bf16 slots in c0

            pairs = []
            for q in range(NDR2):
                rlp = RP.tile([128, 2, N], FP8, tag="rlp", name="rlp")
                for half in range(2):
                    jp = 64 * jt + 2 * q + half
                    eng = pat[_pi[0] % len(pat)]
                    _pi[0] += 1
                    if eng == "a":
                        nc.scalar.activation(out=rlp[:, half, :], in_=Urep[:],
                                             func=AF.Relu,
                                             bias=V2[:, jp:jp + 1], scale=1.0)
                    elif eng == "v":
                        nc.vector.tensor_scalar(out=rlp[:, half, :],
                                                in0=Urep[:],
                                                scalar1=V2[:, jp:jp + 1],
                                                scalar2=0.0, op0=OP.add,
                                                op1=OP.max)
                    else:
                        nc.gpsimd.tensor_scalar(out=rlp[:, half, :],
                                                in0=Urep[:],
                                                scalar1=V2[:, jp:jp + 1],
                                                scalar2=0.0, op0=OP.add,
                                                op1=OP.max)
                pairs.append(rlp)

            def build_bf16(c, s):
                jp = 64 * jt + 32 * c + s
                rl = RL.tile([128, N], BF16, tag="relu", name="rl")
                nc.vector.tensor_scalar(out=rl[:], in0=Urep[:],
                                        scalar1=V2[:, jp:jp + 1],
                                        scalar2=0.0, op0=OP.add, op1=OP.max)
                return rl

            # c0 (DR) and c1 (bf16) accumulate as independent groups on
            # disjoint partition rows of one bank; the sim's zero-region
            # conflict check is coarser than partition rows, so skip it
            n_c0 = NDR2 + C0BF
            mi0 = mi1 = 0
            qn = 0
            for s in range(32):
                if s % 2 == 0 and qn < NDR2:
                    nc.tensor.matmul(ps[ts(0, 64), :], f8w[:, qn],
                                     pairs[qn][:], start=(mi0 == 0),
                                     stop=(mi0 == n_c0 - 1), perf_mode=DR,
                                     skip_group_check=True)
                    mi0 += 1
                    qn += 1
                rl = build_bf16(1, s)
                nc.tensor.matmul(ps[ts(1, 64), :], w2bd_sb[s], rl[:],
                                 start=(mi1 == 0), stop=(mi1 == 31),
                                 skip_group_check=True)
                mi1 += 1
            for q in range(qn, NDR2):
                nc.tensor.matmul(ps[ts(0, 64), :], f8w[:, q], pairs[q][:],
                                 start=(mi0 == 0), stop=(mi0 == n_c0 - 1),
                                 perf_mode=DR, skip_group_check=True)
                mi0 += 1
            for s in range(2 * NDR2, 32):
                rl = build_bf16(0, s)
                nc.tensor.matmul(ps[ts(0, 64), :], w2bd_sb[s], rl[:],
                                 start=(mi0 == 0), stop=(mi0 == n_c0 - 1),
                                 skip_group_check=True)
                mi0 += 1
            if jt == 0:
                emit_sv()
            if jt < NT - 1:
                # exp -> mask -> E^T tile [j, i]
                X = XE.tile([128, N], BF16, tag="X", name="X")
                nc.scalar.activation(out=X[:], in_=ps[:], func=AF.Exp)
                et = P.tile([128, N], BF16, tag=f"ET{jt}", name=f"et{jt}")
                nc.vector.scalar_tensor_tensor(out=et[:], in0=X[:],
                                               scalar=esv[:, jt:jt + 1],
                                               in1=esbs[jt][:], op0=OP.mult,
                                               op1=OP.mult)
                ets.append(et)
            else:
                ps_last = ps

        # ---------- last j-tile chunked by i-half, then agg + output ----------
        jl = NT - 1
        for ih in range(2):
            hb = ts(ih, 256)
            Xc = XE.tile([128, 256], BF16, tag="Xc", name="Xc")
            nc.scalar.activation(out=Xc[:], in_=ps_last[:, hb], func=AF.Exp)
            nc.vector.scalar_tensor_tensor(out=et3[:, hb], in0=Xc[:],
                                           scalar=esv[:, jl:jl + 1],
                                           in1=esbs[jl][:, hb],
                                           op0=OP.mult, op1=OP.mult)
        for it in range(NT):
            ib = ts(it, 128)
            # aggregation (+ denom in ones column), one group at a time
            for jt in range(NT - 1):
                nc.tensor.matmul(pa_all[:, it, :], ets[jt][:, ib],
                                 nbe65[jt][:, 0:H + 1],
                                 start=(jt == 0), stop=False)
            nc.tensor.matmul(pa_all[:, it, :], et3[:, ib],
                             nbe65[jl][:, 0:H + 1], start=False, stop=True)
            # output assembly for this i-tile
            den = pa_all[:, it, H:H + 1]
            gate = SM.tile([128, 1], F32, tag="gate", name="gate")
            nc.vector.tensor_single_scalar(out=gate[:], in_=den, scalar=1e-6,
                                           op=OP.is_gt)
            dsafe = SM.tile([128, 1], F32, tag="dsafe", name="dsafe")
            nc.vector.tensor_scalar_max(out=dsafe[:], in0=den, scalar1=1e-30)
            recipg = SM.tile([128, 1], F32, tag="recipg", name="recipg")
            nc.vector.reciprocal(out=recipg[:], in_=dsafe[:])
            sg = SM.tile([128, H], F32, tag="sg", name="sg")
            nc.vector.tensor_scalar_mul(out=sg[:], in0=selfe[it][:],
                                        scalar1=gate[:])
            ot = SM.tile([128, H], F32, tag="ot", name="ot")
            nc.vector.scalar_tensor_tensor(out=ot[:], in0=pa_all[:, it, 0:H],
                                           scalar=recipg[:], in1=sg[:],
                                           op0=OP.mult, op1=OP.add)
            oeng = [nc.sync, nc.scalar, nc.sync, nc.scalar][it]
            oeng.dma_start(out=t["out"].ap()[ts(it, 128), :], in_=ot[:])


def _prep_edges(edges):
    """uint8 adjacency with the diagonal (self-edges) zeroed host-side."""
    e = (np.asarray(edges) != 0).astype(np.uint8)
    e[:, np.arange(N), np.arange(N)] = 0
    return e


def _host_constants(inputs):
    f32 = np.float32
    bf = ml_dtypes.bfloat16
    f8 = ml_dtypes.float8_e4m3
    H_ = H
    w2 = np.asarray(inputs["comb_w2"], f32)            # [H, 1]
    b1c = np.asarray(inputs["comb_b1"], f32)
    b2s = np.asarray(inputs["self_b2"], f32)
    b2n = np.asarray(inputs["nb_b2"], f32)
    w2s = np.asarray(inputs["self_w2"], f32)
    w2n = np.asarray(inputs["nb_w2"], f32)
    w1cs = np.asarray(inputs["comb_w1"], f32)[:H_]
    w1cn = np.asarray(inputs["comb_w1"], f32)[H_:]
    w2bdpack = np.zeros((128, 2), f32)
    w2bdpack[0:H_, 0] = 0.8 * w2[:, 0]
    w2bdpack[H_:128, 1] = 0.8 * w2[:, 0]
    ndr2 = (64 - 2 * B2G) // 2
    f8lhs = np.zeros((128, max(ndr2, 1), 2, H_), f32)
    for q in range(ndr2):
        for sub in range(2):
            s = 2 * q + sub
            f8lhs[0:H_, q, sub, 2 * s] = 0.8 * w2[:, 0]
            f8lhs[H_:128, q, sub, 2 * s + 1] = 0.8 * w2[:, 0]
    wU = w2s @ w1cs                    # [H, H]
    wV = w2n @ w1cn
    bU = w1cs.T @ b2s                  # [H]
    bV = w1cn.T @ b2n + b1c
    bvec = np.zeros((128, 7), f32)
    bvec[:H_, 0] = np.asarray(inputs["self_b1"], f32)
    bvec[:H_, 1] = np.asarray(inputs["nb_b1"], f32)
    bvec[:H_, 2] = b2s
    bvec[:H_, 3] = b2n
    bvec[:H_, 4] = bU
    bvec[H_:, 4] = bU
    bvec[:H_, 5] = bV
    bvec[H_:, 5] = bV
    bvec[0, 6] = 0.2 * float(w2[:, 0] @ bV)
    wpack1 = np.concatenate([
        np.asarray(inputs["self_w1"], f32),
        np.asarray(inputs["nb_w1"], f32),
        np.eye(128, dtype=f32),
    ], axis=1)
    wpack2 = np.concatenate([wU, wU, wV, wV @ w2, w2s, w2n], axis=1)
    consts = {
        "wpack1": np.ascontiguousarray(wpack1),
        "wpack2": np.ascontiguousarray(wpack2),
        "bvec": bvec,
        "w2bdpack": w2bdpack.astype(bf),
        "f8lhs": f8lhs.astype(f8),
    }
    return consts


def _build_fast_path(nc):
    """Cache a single jitted shard_map executable so repeat kernel() calls
    skip jax re-tracing (same lowering run_bass_kernel_spmd uses under axon)."""
    import jax
    from jax.sharding import Mesh, PartitionSpec
    from jax.experimental.shard_map import shard_map

    bass2jax.install_neuronx_cc_hook()
    pname = nc.partition_id_tensor.name if nc.partition_id_tensor else None
    in_names, out_names, out_avals = [], [], []
    for alloc in nc.m.functions[0].allocations:
        if not isinstance(alloc, mybir.MemoryLocationSet):
            continue
        name = alloc.memorylocations[0].name
        if alloc.kind == "ExternalInput":
            if name != pname:
                in_names.append(name)
        elif alloc.kind == "ExternalOutput":
            out_names.append(name)
            out_avals.append(jax.core.ShapedArray(tuple(alloc.tensor_shape),
                                                  mybir.dt.np(alloc.dtype)))
    all_names = in_names + out_names + ([pname] if pname else [])

    def _body(*args):
        operands = list(args)
        if pname is not None:
            operands.append(bass2jax.partition_id_tensor())
        return tuple(bass2jax._bass_exec_p.bind(
            *operands, out_avals=tuple(out_avals), in_names=tuple(all_names),
            out_names=tuple(out_names), lowering_input_output_aliases=(),
            sim_require_finite=True, sim_require_nnan=True, nc=nc))

    devices = jax.devices()[:NCORES]
    mesh = Mesh(np.asarray(devices), ("core",))
    n_io = len(in_names) + len(out_names)
    sharded = jax.jit(
        shard_map(_body, mesh=mesh, in_specs=(PartitionSpec("core"),) * n_io,
                  out_specs=(PartitionSpec("core"),) * len(out_names),
                  check_rep=False),
        keep_unused=True,
    )
    return sharded, in_names, out_names, out_avals


def kernel(**inputs):
    first = "nc" not in _CACHE
    if first:
        _CACHE["nc"] = _build_module()
    nc = _CACHE["nc"]

    consts = _host_constants(inputs)
    nodes = np.asarray(inputs["nodes"], np.float32).reshape(B, N, D)
    edges = _prep_edges(inputs["edges"])

    in_maps = []
    for c in range(NCORES):
        m = dict(consts)
        m["nodes"] = np.ascontiguousarray(nodes[c])
        m["edges"] = edges[c]
        in_maps.append(m)

    if first:
        res = run_bass_kernel_spmd(nc, in_maps, core_ids=list(range(NCORES)))
        _CACHE["fast"] = _build_fast_path(nc)
        return np.stack([res.results[c]["out"] for c in range(NCORES)]).astype(np.float32)

    import jax
    sharded, in_names, out_names, out_avals = _CACHE["fast"]
    ckey = hash(tuple((k, v.tobytes()) for k, v in sorted(consts.items())))
    if _CACHE.get("ckey") != ckey:
        _CACHE["cdev"] = {
            n: jax.device_put(np.concatenate([np.asarray(in_maps[c][n])
                                              for c in range(NCORES)], axis=0))
            for n in in_names if n not in ("nodes", "edges")
        }
        _CACHE["zdev"] = [jax.device_put(np.zeros((NCORES * a.shape[0], *a.shape[1:]),
                                                  a.dtype)) for a in out_avals]
        _CACHE["ckey"] = ckey
    cdev = _CACHE["cdev"]
    concat_in = [cdev[n] if n in cdev else
                 np.concatenate([np.asarray(in_maps[c][n]) for c in range(NCORES)], axis=0)
                 for n in in_names]
    outs = sharded(*concat_in, *_CACHE["zdev"])
    i = out_names.index("out")
    return np.asarray(outs[i]).reshape(NCORES, N, H).astype(np.float32)
